# revision 1
# baseline (speedup 1.0000x reference)
"""BitLinear (ternary weight quant + per-token int8 activation quant + GEMM)
Trainium2 Bass/Tile kernel, 8-core SPMD.

Sharding: tokens (B*S = 8192) split 8 ways; weight replicated per core.
Each core additionally gets a distinct 512-row slice of W for the global
mean(|W|) partial, combined with a tiny AllReduce.

Math notes (exactness):
  - a_q in [-127,127] and w_q in {-1,0,1} are exact in bf16; the PE
    accumulates fp32 integer partial sums < 2^24, so the GEMM is exact.
  - round-to-nearest-even via the fp32 magic-number trick (+1.5*2^23).
  - clip(round(w/s),-1,1) == sign(round(w/s)) because |w/s| <= 2, so the
    ACT Sign function performs unshift+clip+cast in one op.
"""

import numpy as np

B, S, D = 2, 4096, 4096
NCORES = 8
T = B * S                  # 8192 tokens
TSH = T // NCORES          # 1024 tokens per core
WSL = D // NCORES          # 512 weight rows per core for the mean partial
P = 128
MAGIC = 1.5 * 2**23        # 12582912.0; forces RNE-to-integer in fp32
EPS = 1e-8
QMAX = 127.0
NELEM = float(D * D)

_CACHE: dict = {}


def _build(reps=1, variant='full'):
    import concourse.bass as bass
    import concourse.mybir as mybir
    import concourse.tile as tile
    from concourse import bacc
    from concourse.masks import make_identity

    f32 = mybir.dt.float32
    bf16 = mybir.dt.bfloat16
    X = mybir.AxisListType.X

    nc = bacc.Bacc(
        "TRN2", target_bir_lowering=False, debug=False, num_devices=NCORES
    )

    xs = nc.dram_tensor("xs", [TSH, D], f32, kind="ExternalInput").ap()
    # The full weight is only read by variants that quantize slices locally;
    # declaring it otherwise would force a useless 67MB/core host transfer.
    need_w = variant in ("full", "ag2")
    w = (
        nc.dram_tensor("w", [D, D], f32, kind="ExternalInput").ap()
        if need_w
        else None
    )
    wslice = nc.dram_tensor("wslice", [WSL, D], f32, kind="ExternalInput").ap()
    y = nc.dram_tensor("y", [TSH, D], f32, kind="ExternalOutput").ap()

    NT = TSH // P      # 8 token tiles
    NI = D // P        # 32 contraction blocks
    NS = NCORES        # 8 output slices of 512
    OSL = D // NS      # 512 output cols per slice
    NC_W = OSL // P    # 4 weight row-chunks per slice

    with tile.TileContext(nc) as tc:
        with (
            tc.tile_pool(name="stage", bufs=2 if variant == "agb" else 3) as stage,
            tc.tile_pool(name="xqt", bufs=1) as xqt_pool,
            tc.tile_pool(name="wqt", bufs=2) as wqt_pool,
            tc.tile_pool(name="small", bufs=1) as small,
            tc.tile_pool(name="ysb", bufs=3) as ysb_pool,
            tc.tile_pool(name="xb", bufs=2) as xb_pool,
            tc.tile_pool(name="pt", bufs=2, space="PSUM") as psum_t,
            tc.tile_pool(name="py", bufs=4, space="PSUM") as psum_y,
            tc.tile_pool(name="dram", bufs=1, space="DRAM") as dram,
        ):
            ident = small.tile([P, P], f32, tag="ident")
            make_identity(nc, ident)
            if variant == 'agb':
                ident_b = small.tile([P, P], bf16, tag="ident_b")
                make_identity(nc, ident_b)
            negm = small.tile([P, 1], f32, tag="negm")
            nc.vector.memset(negm, -MAGIC)

            # ---- Phase A: partial sum of |wslice|, AllReduce -> w_scale ----
            def phase_a():
                partials = small.tile([P, 4], f32, tag="partials")
                for c in range(WSL // P):
                    st = stage.tile([P, D], f32, tag="stage")
                    nc.sync.dma_start(st, wslice[c * P:(c + 1) * P, :])
                    t8 = small.tile([P, 8], f32, tag="t8")
                    nc.vector.tensor_reduce(
                        t8, st.rearrange("p (a b) -> p a b", b=512), axis=X,
                        op=mybir.AluOpType.add, apply_absolute_value=True,
                    )
                    nc.vector.reduce_sum(partials[:, c:c + 1], t8, axis=X)
                pcol = small.tile([P, 1], f32, tag="pcol")
                nc.vector.reduce_sum(pcol, partials, axis=X)

                bounce_in = dram.tile([P, 1], f32, tag="cc_in")
                bounce_out = dram.tile([P, 1], f32, tag="cc_out")
                nc.sync.dma_start(bounce_in, pcol)
                nc.gpsimd.collective_compute(
                    "AllReduce",
                    mybir.AluOpType.add,
                    replica_groups=[list(range(NCORES))],
                    ins=[bounce_in.opt()],
                    outs=[bounce_out.opt()],
                )
                srow = small.tile([1, P], f32, tag="srow")
                nc.sync.dma_start(srow, bounce_out.rearrange("p one -> one p"))
                stot = small.tile([1, 1], f32, tag="stot")
                nc.vector.reduce_sum(stot, srow, axis=X)
                # w_scale = mean + EPS ; also 1/w_scale and w_scale/127
                ws = small.tile([1, 1], f32, tag="ws")
                nc.vector.tensor_scalar(
                    ws, stot, 1.0 / NELEM, EPS,
                    op0=mybir.AluOpType.mult, op1=mybir.AluOpType.add,
                )
                wr = small.tile([1, 1], f32, tag="wr")
                nc.vector.reciprocal(wr, ws)
                w127 = small.tile([1, 1], f32, tag="w127")
                nc.vector.tensor_scalar_mul(w127, ws, 1.0 / QMAX)
                wr_col = small.tile([P, 1], f32, tag="wr_col")
                nc.gpsimd.partition_broadcast(wr_col, wr)
                w127_col = small.tile([P, 1], f32, tag="w127_col")
                nc.gpsimd.partition_broadcast(w127_col, w127)
                return wr_col, w127_col

            wr_col, w127_col = (None, None) if variant == 'all_in' else phase_a()

            def phase_a2(wr_col):
                # ---- Phase A2: quantize own W slice, AllGather ----
                wqo = wqt_pool.tile([P, NI, OSL], bf16, tag="wqt")
                for c in range(NC_W):
                    st = stage.tile([P, D], f32, tag="stage")
                    nc.sync.dma_start(st, wslice[c * P:(c + 1) * P, :])
                    nc.vector.tensor_scalar(
                        st, st, wr_col, MAGIC,
                        op0=mybir.AluOpType.mult,
                        op1=mybir.AluOpType.add,
                    )
                    for g in range(NI // 8):
                        ps = psum_t.tile([P, 1024], f32, tag="pt")
                        for bq in range(8):
                            ib = g * 8 + bq
                            nc.tensor.matmul(
                                ps[:, bq * P:(bq + 1) * P],
                                lhsT=st[:, ib * P:(ib + 1) * P],
                                rhs=ident,
                                start=True, stop=True,
                            )
                        nc.scalar.activation(
                            wqo[:, g * 8:g * 8 + 8, c * P:(c + 1) * P],
                            ps.rearrange("p (a b) -> p a b", b=P),
                            mybir.ActivationFunctionType.Sign,
                            bias=negm,
                        )
                ag_in = dram.tile([NI, P, OSL], bf16, tag="ag_in")
                nc.sync.dma_start(ag_in.rearrange("b p o -> p b o"), wqo)
                ag_out = dram.tile(
                    [NCORES, NI, P, OSL], bf16, tag="ag_out",
                    addr_space="Shared",
                )
                nc.gpsimd.collective_compute(
                    "AllGather",
                    mybir.AluOpType.bypass,
                    replica_groups=[list(range(NCORES))],
                    ins=[ag_in.opt()],
                    outs=[ag_out.opt()],
                )
                return ag_out

            ag_out = phase_a2(wr_col) if variant in ('ag', 'ag2', 'agb') else None

            # ---- Phases B-D (optionally repeated for benchmarking) ----
            def body():
                if variant == 'all_in':
                    wrc, w127c = phase_a()
                    ag_o = phase_a2(wrc)
                else:
                    wrc, w127c = wr_col, w127_col
                    ag_o = phase_a2(wr_col) if variant == 'ag_all' else ag_out
                # Phase B: x quant (+shift), transpose -> xqT bf16 [i, t]
                xqT = xqt_pool.tile([P, NI, TSH], bf16, tag="xqt")
                comb = small.tile([P, NT], f32, tag="comb")  # a_scale*w_scale/127
                if variant == 'mm_only':
                    nc.vector.memset(xqT, 1.0)
                    nc.vector.memset(comb, 1.0)
                for t in range(0 if variant == 'mm_only' else NT):
                    st = stage.tile([P, D], f32, tag="stage")
                    nc.sync.dma_start(st, xs[t * P:(t + 1) * P, :])
                    amax = small.tile([P, 1], f32, tag="amax")
                    nc.vector.tensor_reduce(
                        amax, st, axis=X, op=mybir.AluOpType.max,
                        apply_absolute_value=True,
                    )
                    a_scale = small.tile([P, 1], f32, tag="a_scale")
                    nc.vector.tensor_scalar_add(a_scale, amax, EPS)
                    arec = small.tile([P, 1], f32, tag="arec")
                    nc.vector.reciprocal(arec, a_scale)
                    r127 = small.tile([P, 1], f32, tag="r127")
                    nc.vector.tensor_scalar_mul(r127, arec, QMAX)
                    nc.vector.tensor_scalar(
                        comb[:, t:t + 1], a_scale, w127c, None,
                        op0=mybir.AluOpType.mult,
                    )
                    # in-place: st <- st * r127 + MAGIC  (RNE to integer + shift)
                    nc.vector.tensor_scalar(
                        st, st, r127, MAGIC,
                        op0=mybir.AluOpType.mult, op1=mybir.AluOpType.add,
                    )
                    if variant == 'agb':
                        # unshift to bf16 first: transposes then load weights
                        # at FWL (2 elem/cycle) instead of fp32 1 elem/cycle
                        xb = xb_pool.tile([P, D], bf16, tag="xb")
                        nc.vector.tensor_scalar_sub(xb, st, MAGIC)
                        for g in range(NI // 8):
                            ps = psum_t.tile([P, 1024], f32, tag="pt")
                            for bq in range(8):
                                ib = g * 8 + bq
                                nc.tensor.matmul(
                                    ps[:, bq * P:(bq + 1) * P],
                                    lhsT=xb[:, ib * P:(ib + 1) * P],
                                    rhs=ident_b,
                                    start=True, stop=True,
                                )
                            nc.scalar.activation(
                                xqT[:, g * 8:g * 8 + 8, t * P:(t + 1) * P],
                                ps.rearrange("p (a b) -> p a b", b=P),
                                mybir.ActivationFunctionType.Copy,
                            )
                    else:
                        for g in range(NI // 8):
                            ps = psum_t.tile([P, 1024], f32, tag="pt")
                            for bq in range(8):
                                ib = g * 8 + bq
                                nc.tensor.matmul(
                                    ps[:, bq * P:(bq + 1) * P],
                                    lhsT=st[:, ib * P:(ib + 1) * P],
                                    rhs=ident,
                                    start=True, stop=True,
                                )
                            # unshift + cast to bf16
                            nc.scalar.activation(
                                xqT[:, g * 8:g * 8 + 8, t * P:(t + 1) * P],
                                ps.rearrange("p (a b) -> p a b", b=P),
                                mybir.ActivationFunctionType.Identity,
                                bias=negm,
                            )

                # ---- Phase C/D: per output slice: quantize W rows, GEMM ----
                for s in range(NS):
                    wqT = wqt_pool.tile([P, NI, OSL], bf16, tag="wqt")
                    local_quant = variant == 'full' or (
                        variant == 'ag2' and s < 2
                    )
                    if variant in ('gemm_only', 'mm_only'):
                        nc.vector.memset(wqT, 1.0)
                    if variant in ('ag', 'agb', 'ag_all', 'all_in') or (
                        variant == 'ag2' and not local_quant
                    ):
                        nc.sync.dma_start(
                            wqT, ag_o[s].rearrange("b p o -> p b o")
                        )
                    w_chunks = NC_W if local_quant else 0
                    for c in range(w_chunks):
                        st = stage.tile([P, D], f32, tag="stage")
                        nc.sync.dma_start(
                            st, w[s * OSL + c * P: s * OSL + (c + 1) * P, :]
                        )
                        # in-place: st <- st * (1/w_scale) + MAGIC
                        nc.vector.tensor_scalar(
                            st, st, wrc, MAGIC,
                            op0=mybir.AluOpType.mult, op1=mybir.AluOpType.add,
                        )
                        for g in range(NI // 8):
                            ps = psum_t.tile([P, 1024], f32, tag="pt")
                            for bq in range(8):
                                ib = g * 8 + bq
                                nc.tensor.matmul(
                                    ps[:, bq * P:(bq + 1) * P],
                                    lhsT=st[:, ib * P:(ib + 1) * P],
                                    rhs=ident,
                                    start=True, stop=True,
                                )
                            # sign(v - MAGIC) == clip(round(w/s), -1, 1); bf16 out
                            nc.scalar.activation(
                                wqT[:, g * 8:g * 8 + 8, c * P:(c + 1) * P],
                                ps.rearrange("p (a b) -> p a b", b=P),
                                mybir.ActivationFunctionType.Sign,
                                bias=negm,
                            )
                    for t in range(NT):
                        py = psum_y.tile([P, OSL], f32, tag="py")
                        mm_iters = 1 if variant == 'no_mm' else NI
                        for i in range(mm_iters):
                            nc.tensor.matmul(
                                py,
                                lhsT=xqT[:, i, t * P:(t + 1) * P],
                                rhs=wqT[:, i, :],
                                start=(i == 0),
                                stop=(i == mm_iters - 1),
                            )
                        yt = ysb_pool.tile([P, OSL], f32, tag="ysb")
                        nc.scalar.mul(yt, py, comb[:, t:t + 1])
                        nc.sync.dma_start(
                            y[t * P:(t + 1) * P, s * OSL:(s + 1) * OSL], yt
                        )

            if reps == 1:
                body()
            else:
                with tc.For_i(0, reps, 1):
                    body()

    nc.compile()
    return nc


def _get_nc(reps=1, variant='full'):
    key = f"nc{reps}-{variant}"
    if key not in _CACHE:
        _CACHE[key] = _build(reps, variant)
    return _CACHE[key]


def run(x, weight, trace=False, variant="ag", reps=1):
    from concourse.bass_utils import run_bass_kernel_spmd

    nc = _get_nc(reps, variant)
    x = np.ascontiguousarray(np.asarray(x, dtype=np.float32))
    weight = np.ascontiguousarray(np.asarray(weight, dtype=np.float32))
    xf = x.reshape(T, D)
    in_maps = []
    for c in range(NCORES):
        m = {
            "xs": xf[c * TSH:(c + 1) * TSH],
            "wslice": weight[c * WSL:(c + 1) * WSL],
        }
        if variant in ("full", "ag2"):
            m["w"] = weight
        in_maps.append(m)
    res = run_bass_kernel_spmd(
        nc, in_maps, core_ids=list(range(NCORES)), trace=trace
    )
    yf = np.concatenate([res.results[c]["y"] for c in range(NCORES)], axis=0)
    return yf.reshape(B, S, D), res


def kernel(x, weight):
    out, _ = run(x, weight, trace=False)
    return out

